# revision 3
# baseline (speedup 1.0000x reference)
"""Causal multi-head self-attention with RoPE on 8 Trainium2 NeuronCores.

Sharding: core = (batch b, head-group g) with b = core//2, g = core%2.
Each core computes QKV projections for its batch element restricted to its
8 heads (512 of 1024 projection rows), RoPE, causal attention, and the
partial output projection y_g = attn_g @ wo[:, g*512:(g+1)*512].T.  The host
sums the two head-group partials per batch element.

Layout: scores are computed TRANSPOSED (k on partitions, q on the free
axis): exp() is elementwise, the denominator comes from M=1 ones-matmuls
(col-tiled into spare PE array columns), and exp(scores^T) feeds the PV
matmul directly as the moving operand.

Performance structure (v2):
 - Score matmuls for a head PAIR are row-tiled (tile_position (0,0)/(64,0)),
   contracting both 64-dim heads in one 128-partition moving stream: 2x.
 - PV matmuls for the pair are col-tiled ((0,0)/(0,64)) into one PSUM bank.
 - Softmax denominators accumulate in a PSUM bank via M=1 matmuls of a ones
   column against ex, 4-way col-tiled two-kb-at-a-time (parity rows).
 - exp() runs 1024 wide across the pair's two score banks in one ACT call.
 - The causal diagonal mask is applied AFTER exp by gpsimd affine_select
   (zero-fill) on the SBUF ex tile, keeping DVE out of the attention loop.
 - Projections/RoPE/transposes for s-block group g+1 are interleaved
   chunk-by-chunk with attention for q-group g, so the PE never idles while
   the ACT engine streams exps (keeps the HAM clock-gate at full rate) and
   the output projection of q-group g-1 fills the PE during later groups.
"""
import math
import os
from contextlib import ExitStack

import numpy as np
import ml_dtypes

import concourse.bass as bass
import concourse.tile as tile
from concourse import bacc, mybir
from concourse import masks
from concourse.bass_utils import run_bass_kernel_spmd

F32 = mybir.dt.float32
BF16 = mybir.dt.bfloat16

D = 1024          # d_model
NH = 16           # heads total
DK = 64           # head dim
S = 2048          # sequence
B = 4             # batch
THETA = 10000.0
HPG = 8           # heads per group (2 groups over 8 cores with 4 batches)
W = HPG * DK      # 512: local projection width
NSB = S // 128    # 16 s-blocks
NQG = 4           # 512-wide q groups
SCALE = 1.0 / math.sqrt(DK)

TRACE = bool(int(os.environ.get("KTRACE", "0")))

_cache = {}


def build_nc():
    sdt = BF16
    nc = bacc.Bacc(None, target_bir_lowering=False, debug=False)

    xt = nc.dram_tensor("xt", [D, S], sdt, kind="ExternalInput")
    wqt = nc.dram_tensor("wqt", [D, W], sdt, kind="ExternalInput")
    wkt = nc.dram_tensor("wkt", [D, W], sdt, kind="ExternalInput")
    wvt = nc.dram_tensor("wvt", [D, W], sdt, kind="ExternalInput")
    wot = nc.dram_tensor("wot", [W, D], sdt, kind="ExternalInput")
    cosb = nc.dram_tensor("cosb", [S, W // 2], F32, kind="ExternalInput")
    sinb = nc.dram_tensor("sinb", [S, W // 2], F32, kind="ExternalInput")
    yp = nc.dram_tensor("yp", [S, D], F32, kind="ExternalOutput")

    xt3 = xt[:].rearrange("(jo p) s -> p jo s", p=128)       # [128, 8, S]
    wqt3 = wqt[:].rearrange("(jo p) i -> p jo i", p=128)     # [128, 8, W]
    wkt3 = wkt[:].rearrange("(jo p) i -> p jo i", p=128)
    wvt3 = wvt[:].rearrange("(jo p) i -> p jo i", p=128)
    wot3 = wot[:].rearrange("(jo p) i -> p jo i", p=128)     # [128, 4, D]

    with tile.TileContext(nc, pool_alloc_mode="queue") as tc, \
            ExitStack() as ctx:
        persist = ctx.enter_context(tc.tile_pool(name="persist", bufs=1))
        ident = persist.tile([128, 128], F32, name="ident")
        masks.make_identity(nc, ident)
        ones = persist.tile([128, 1], sdt, name="ones")
        nc.gpsimd.memset(ones, 1.0)

        # persistent activations: q^T and k^T as 4 head-pair slabs
        # (rows 0:64 = head 2j dims [evens|odds], rows 64:128 = head 2j+1),
        # v s-major [128, 8*64].
        qT = [persist.tile([128, S], sdt, name=f"qT{j}") for j in range(4)]
        kT = [persist.tile([128, S], sdt, name=f"kT{j}") for j in range(4)]
        vt = [persist.tile([128, W], sdt, name=f"vt{i}") for i in range(NSB)]

        wp = ctx.enter_context(tc.tile_pool(name="wp", bufs=1))
        wq_s = wp.tile([128, 8, W], sdt, name="wq_s")
        nc.sync.dma_start(wq_s[:], wqt3[:])
        wk_s = wp.tile([128, 8, W], sdt, name="wk_s")
        wv_s = wp.tile([128, 8, W], sdt, name="wv_s")
        nc.gpsimd.dma_start(wk_s[:], wkt3[:])
        nc.gpsimd.dma_start(wv_s[:], wvt3[:])
        wo_s = wp.tile([128, 4, D], sdt, name="wo_s")
        nc.gpsimd.dma_start(wo_s[:], wot3[:])

        px = ctx.enter_context(tc.tile_pool(name="px", bufs=2))
        rp = ctx.enter_context(tc.tile_pool(name="rp", bufs=2))
        exp_p = ctx.enter_context(tc.tile_pool(name="exp_p", bufs=4))
        aqp = ctx.enter_context(tc.tile_pool(name="aqp", bufs=2))
        nrm = ctx.enter_context(tc.tile_pool(name="nrm", bufs=2))
        ytp = ctx.enter_context(tc.tile_pool(name="ytp", bufs=2))
        # PSUM: pp(1) + tr(1) + sc(2x2) + pv(1) + den(1) = 8 banks
        pp = ctx.enter_context(tc.tile_pool(name="pp", bufs=1, space="PSUM"))
        trp = ctx.enter_context(tc.tile_pool(name="trp", bufs=1, space="PSUM"))
        scp = ctx.enter_context(tc.tile_pool(name="scp", bufs=2, space="PSUM"))
        pvp = ctx.enter_context(tc.tile_pool(name="pvp", bufs=1, space="PSUM"))
        dnp = ctx.enter_context(tc.tile_pool(name="dnp", bufs=1, space="PSUM"))

        def rope(ps, outt, c3, s3):
            # ps: [128, W] PSUM (pre-RoPE proj, s-major, heads as
            # [evens(32) | odds(32)] blocks); outt: [128, W] SBUF
            pe = ps.rearrange("p (h eo c) -> p h eo c", eo=2, c=32)
            ein, oin = pe[:, :, 0, :], pe[:, :, 1, :]
            oe = outt.rearrange("p (h eo c) -> p h eo c", eo=2, c=32)
            eout, oout = oe[:, :, 0, :], oe[:, :, 1, :]
            ra = rp.tile([128, 8, 32], F32, name="ra", tag="ra")
            rb = rp.tile([128, 8, 32], F32, name="rb", tag="rb")
            nc.vector.tensor_mul(ra, ein, c3)
            nc.vector.tensor_mul(rb, oin, s3)
            nc.vector.tensor_sub(eout, ra, rb)
            rc = rp.tile([128, 8, 32], F32, name="rc", tag="rc")
            rd = rp.tile([128, 8, 32], F32, name="rd", tag="rd")
            nc.vector.tensor_mul(rc, ein, s3)
            nc.vector.tensor_mul(rd, oin, c3)
            nc.vector.tensor_add(oout, rc, rd)

        # ---------------- proj sub-chunk emitters -------------------------
        def proj_load(sb):
            s0 = sb * 128
            xs = px.tile([128, 8, 128], sdt, name="xs", tag="xs")
            nc.sync.dma_start(xs[:], xt3[:, :, s0:s0 + 128])
            cs = px.tile([128, W // 2], F32, name="cs", tag="cs")
            nc.sync.dma_start(cs[:], cosb[s0:s0 + 128, :])
            sn = px.tile([128, W // 2], F32, name="sn", tag="sn")
            nc.sync.dma_start(sn[:], sinb[s0:s0 + 128, :])
            return xs, cs, sn

        def proj_qk(which, sb, st):
            xs, cs, sn = st
            c3 = cs.rearrange("p (h c) -> p h c", c=32)
            s3 = sn.rearrange("p (h c) -> p h c", c=32)
            wsb = wq_s if which == "q" else wk_s
            pj = pp.tile([128, W], F32, name="pj", tag="pp")
            for jo in range(8):
                nc.tensor.matmul(pj[:], xs[:, jo, :], wsb[:, jo, :],
                                 start=(jo == 0), stop=(jo == 7))
            ro = rp.tile([128, W], F32, name=f"{which}_ro", tag=f"{which}ro")
            rope(pj, ro, c3, s3)
            return ro

        def proj_v(sb, st):
            xs, _, _ = st
            pj = pp.tile([128, W], F32, name="pj", tag="pp")
            for jo in range(8):
                nc.tensor.matmul(pj[:], xs[:, jo, :], wv_s[:, jo, :],
                                 start=(jo == 0), stop=(jo == 7))
            nc.scalar.copy(vt[sb][:], pj[:])

        def transp2(sb, pr, q_ro, k_ro):
            # transpose head-pair pr of q_ro and k_ro into the d-major slabs
            s0 = sb * 128
            c0 = pr * 128
            for srcb, dstl in ((q_ro, qT), (k_ro, kT)):
                ptr = trp.tile([128, 128], F32, name="ptr", tag="tr")
                nc.tensor.transpose(ptr[:], srcb[:, c0:c0 + 128], ident[:])
                nc.vector.tensor_copy(dstl[pr][:, s0:s0 + 128], ptr[:])

        # ---------------- attention chunk emitters ------------------------
        pair_state = {}

        def attn_pair_begin(qg, p):
            pvAB = pvp.tile([128, 512], F32, name="pvAB", tag="pv")
            den = dnp.tile([128, 512], F32, name="den", tag="den")
            pair_state[(qg, p)] = (pvAB, den, [])

        def attn_chunk(qg, p, kb):
            pvAB, den, exs = pair_state[(qg, p)]
            nkb = 4 * qg + 4
            q0 = 512 * qg
            off = kb - 4 * qg
            c0 = 128 * max(off, 0)
            scAB = scp.tile([128, 1024], F32, name="scAB", tag="sc")
            nc.tensor.matmul(
                scAB[:, c0:512],
                kT[p][0:64, kb * 128:(kb + 1) * 128],
                qT[p][0:64, q0 + c0:q0 + 512],
                start=True, stop=True, tile_position=(0, 0))
            nc.tensor.matmul(
                scAB[:, 512 + c0:1024],
                kT[p][64:128, kb * 128:(kb + 1) * 128],
                qT[p][64:128, q0 + c0:q0 + 512],
                start=True, stop=True, tile_position=(64, 0))
            ex = exp_p.tile([128, 1024], sdt, name="ex", tag="ex")
            nc.scalar.activation(ex[:], scAB[:],
                                 mybir.ActivationFunctionType.Exp,
                                 scale=SCALE)
            if off >= 0:
                # zero the strictly-upper (k > q) triangle of the diagonal
                # 128-block of both heads, post-exp
                for base in (c0, 512 + c0):
                    nc.gpsimd.affine_select(
                        out=ex[:, base:base + 128], in_=ex[:, base:base + 128],
                        compare_op=mybir.AluOpType.is_ge, fill=0.0,
                        base=0, pattern=[[1, 128]], channel_multiplier=-1)
            nc.tensor.matmul(
                pvAB[0:64, c0:512], vt[kb][:, 128 * p:128 * p + 64],
                ex[:, c0:512],
                start=(kb == 0), stop=(kb == nkb - 1), tile_position=(0, 0))
            nc.tensor.matmul(
                pvAB[64:128, c0:512], vt[kb][:, 128 * p + 64:128 * p + 128],
                ex[:, 512 + c0:1024],
                start=(kb == 0), stop=(kb == nkb - 1), tile_position=(0, 64))
            exs.append((ex, c0))
            # denominators: M=1 ones-matmuls, col-tiled.
            if qg == 0:
                nc.tensor.matmul(den[0:1, c0:512], ones[:], ex[:, c0:512],
                                 start=(kb == 0), stop=(kb == nkb - 1),
                                 tile_position=(0, 0))
                nc.tensor.matmul(den[32:33, c0:512], ones[:],
                                 ex[:, 512 + c0:1024],
                                 start=(kb == 0), stop=(kb == nkb - 1),
                                 tile_position=(0, 32))
            elif kb % 2 == 1:
                exp_, cp = exs[-2]
                nc.tensor.matmul(den[0:1, cp:512], ones[:], exp_[:, cp:512],
                                 start=(kb == 1), stop=(kb == nkb - 1),
                                 tile_position=(0, 0))
                nc.tensor.matmul(den[32:33, cp:512], ones[:],
                                 exp_[:, 512 + cp:1024],
                                 start=(kb == 1), stop=(kb == nkb - 1),
                                 tile_position=(0, 32))
                nc.tensor.matmul(den[64:65, c0:512], ones[:], ex[:, c0:512],
                                 start=(kb == 1), stop=(kb == nkb - 1),
                                 tile_position=(0, 64))
                nc.tensor.matmul(den[96:97, c0:512], ones[:],
                                 ex[:, 512 + c0:1024],
                                 start=(kb == 1), stop=(kb == nkb - 1),
                                 tile_position=(0, 96))

        def attn_pair_end(qg, p, stage8):
            pvAB, den, exs = pair_state.pop((qg, p))
            aq = aqp.tile([128, 512], sdt, name="aq", tag=f"aq{p}")
            nc.vector.tensor_copy(aq[:], pvAB[:])
            pair_state[("aq", qg, p)] = aq
            if qg == 0:
                for row, h in ((0, 0), (32, 1)):
                    dsb = nrm.tile([1, 512], F32, name="dsb", tag="dsb",
                                   bufs=4)
                    nc.vector.tensor_copy(dsb[:], den[row:row + 1, :])
                    nc.sync.dma_start(stage8[2 * p + h:2 * p + h + 1, :],
                                      dsb[:])
            else:
                for row, h in ((0, 0), (32, 1)):
                    tA = nrm.tile([1, 512], F32, name="tA", tag="tA", bufs=4)
                    nc.vector.tensor_copy(tA[:], den[row + 64:row + 65, :])
                    dsb = nrm.tile([1, 512], F32, name="dsb", tag="dsb",
                                   bufs=4)
                    nc.vector.tensor_add(dsb[:], den[row:row + 1, :], tA[:])
                    nc.sync.dma_start(stage8[2 * p + h:2 * p + h + 1, :],
                                      dsb[:])

        def attn_norm(qg, stage8):
            rall8 = nrm.tile([8, 512], F32, name="rall8", tag="rall8")
            nc.vector.reciprocal(rall8[:], stage8[:])
            for h in range(HPG):
                p, r0 = h // 2, 64 * (h % 2)
                rsb = nrm.tile([1, 512], F32, name="rsb", tag="rsb", bufs=3)
                nc.sync.dma_start(rsb[:], rall8[h:h + 1, :])
                rbc = nrm.tile([128, 512], F32, name="rbc", tag="rbc",
                               bufs=3)
                nc.gpsimd.partition_broadcast(rbc[:], rsb[:], channels=128)
                aq = pair_state[("aq", qg, p)]
                nc.vector.tensor_mul(aq[r0:r0 + 64, :], aq[r0:r0 + 64, :],
                                     rbc[r0:r0 + 64, :])

        def outproj_half(qg, sbl, ih, yt):
            s0 = 512 * qg + sbl * 128
            py = trp.tile([128, 512], F32, name="py", tag="tr")
            for j in range(4):
                aq = pair_state[("aq", qg, j)]
                nc.tensor.matmul(py[:], aq[:, sbl * 128:(sbl + 1) * 128],
                                 wo_s[:, j, ih * 512:(ih + 1) * 512],
                                 start=(j == 0), stop=(j == 3))
            nc.vector.tensor_copy(yt[:, ih * 512:(ih + 1) * 512], py[:])
            if ih == 1:
                nc.sync.dma_start(yp[s0:s0 + 128, :], yt[:])

        # ---------------- merged schedule ---------------------------------
        def proj_items(g):
            items = []
            for sb in range(4 * g, 4 * g + 4):
                st_box = {}

                def load(sb=sb, st_box=st_box):
                    st_box["st"] = proj_load(sb)
                    st_box["q"] = proj_qk("q", sb, st_box["st"])
                items.append(load)
                def kproj(sb=sb, st_box=st_box):
                    st_box["k"] = proj_qk("k", sb, st_box["st"])
                items.append(kproj)
                items.append(lambda sb=sb, st_box=st_box:
                             proj_v(sb, st_box["st"]))
                for pr in range(4):
                    items.append(lambda sb=sb, pr=pr, st_box=st_box:
                                 transp2(sb, pr, st_box["q"], st_box["k"]))
            return items

        def outproj_items(qg):
            items = []
            for sbl in range(4):
                yt_box = {}

                def mk(qg=qg, sbl=sbl, ih=0, yt_box=yt_box):
                    yt_box["yt"] = ytp.tile([128, D], F32, name="yt",
                                            tag="yt")
                    outproj_half(qg, sbl, ih, yt_box["yt"])
                items.append(mk)
                items.append(lambda qg=qg, sbl=sbl, yt_box=yt_box:
                             outproj_half(qg, sbl, 1, yt_box["yt"]))
            return items

        def attn_items(qg, stage8):
            items = []
            nkb = 4 * qg + 4
            for p in range(4):
                items.append(lambda qg=qg, p=p: attn_pair_begin(qg, p))
                for kb in range(nkb):
                    items.append(lambda qg=qg, p=p, kb=kb:
                                 attn_chunk(qg, p, kb))
                items.append(lambda qg=qg, p=p, stage8=stage8:
                             attn_pair_end(qg, p, stage8))
            items.append(lambda qg=qg, stage8=stage8: attn_norm(qg, stage8))
            return items

        def interleave(main, filler):
            # spread filler items evenly through main items
            if not main:
                return filler
            out = []
            nf, nm = len(filler), len(main)
            fi = 0
            for i, m in enumerate(main):
                out.append(m)
                want = (i + 1) * nf // nm
                while fi < want:
                    out.append(filler[fi])
                    fi += 1
            out.extend(filler[fi:])
            return out

        stage_tiles = {}
        for g in range(5):
            filler = []
            if g < 4:
                filler.extend(proj_items(g))
            if g >= 2:
                filler.extend(outproj_items(g - 2))
            if g >= 1:
                qg = g - 1
                stage_tiles[qg] = nrm.tile([8, 512], F32,
                                           name=f"stage8_{qg}",
                                           tag="stage8")
                items = interleave(attn_items(qg, stage_tiles[qg]), filler)
            else:
                items = filler
            for it in items:
                it()
        for it in outproj_items(3):
            it()

    nc.compile()
    return nc


def _prep_inputs(x, token_positions, wq, wk, wv, wo):
    hdt = ml_dtypes.bfloat16
    # per-head permutation: [0,2,...,62, 1,3,...,63] (evens then odds)
    pi = np.concatenate([np.arange(0, DK, 2), np.arange(1, DK, 2)])
    perm = (np.arange(NH)[:, None] * DK + pi[None, :]).reshape(-1)
    wq_p = wq[perm, :]
    wk_p = wk[perm, :]

    pos = np.asarray(token_positions).astype(np.float32)
    thetas = (1.0 / (THETA ** (2.0 * np.arange(DK // 2, dtype=np.float32)
                               / DK))).astype(np.float32)
    ang = np.outer(pos, thetas).astype(np.float32)          # [S, 32]
    cos = np.tile(np.cos(ang), (1, HPG)).astype(np.float32)  # [S, 256]
    sin = np.tile(np.sin(ang), (1, HPG)).astype(np.float32)

    in_maps = []
    for core in range(8):
        b, g = core // 2, core % 2
        gs = slice(g * W, (g + 1) * W)
        in_maps.append({
            "xt": np.ascontiguousarray(x[b].T).astype(hdt),
            "wqt": np.ascontiguousarray(wq_p[gs, :].T).astype(hdt),
            "wkt": np.ascontiguousarray(wk_p[gs, :].T).astype(hdt),
            "wvt": np.ascontiguousarray(wv[gs, :].T).astype(hdt),
            "wot": np.ascontiguousarray(wo[:, gs].T).astype(hdt),
            "cosb": cos,
            "sinb": sin,
        })
    return in_maps


last_exec_time_ns = None
MODE = "bf16"


def _install_ntff_hook_shim():
    """This image's antenv lacks axon_hooks; wire the ctypes NTFF hook from
    trn_agent_boot so trace=True yields HW exec times."""
    import sys as _sys
    import types as _types
    try:
        from antenv import axon_hooks  # noqa: F401
        return
    except ImportError:
        pass
    from trn_agent_boot.trn_boot import _ntff_profile_via_ctypes
    hook = _ntff_profile_via_ctypes("/opt/axon/libaxon_pjrt.so")
    mod = _types.ModuleType("antenv.axon_hooks")
    mod.get_axon_ntff_profile_hook = lambda: hook
    _sys.modules["antenv.axon_hooks"] = mod


def kernel(x, token_positions, wq, wk, wv, wo):
    global last_exec_time_ns
    x = np.asarray(x, dtype=np.float32)
    token_positions = np.asarray(token_positions)
    wq = np.asarray(wq, dtype=np.float32)
    wk = np.asarray(wk, dtype=np.float32)
    wv = np.asarray(wv, dtype=np.float32)
    wo = np.asarray(wo, dtype=np.float32)

    if "nc" not in _cache:
        _cache["nc"] = build_nc()
    nc = _cache["nc"]

    in_maps = _prep_inputs(x, token_positions, wq, wk, wv, wo)
    res = None
    if TRACE:
        try:
            _install_ntff_hook_shim()
            res = run_bass_kernel_spmd(nc, in_maps, list(range(8)),
                                       trace=True,
                                       trace_cores=list(range(8)))
        except Exception as e:  # profiling must never sink correctness
            print(f"trace run failed ({type(e).__name__}: {e}); "
                  f"retrying untraced")
            res = None
    if res is None:
        res = run_bass_kernel_spmd(nc, in_maps, list(range(8)))
    last_exec_time_ns = res.exec_time_ns

    out = np.empty((B, S, D), dtype=np.float32)
    for b in range(B):
        out[b] = res.results[2 * b]["yp"] + res.results[2 * b + 1]["yp"]
    return out


# revision 10
# speedup vs baseline: 1.1878x; 1.1878x over previous
"""Causal multi-head self-attention with RoPE on 8 Trainium2 NeuronCores.

Sharding: core = (batch b, head-group g) with b = core//2, g = core%2.
Each core computes QKV projections for its batch element restricted to its
8 heads (512 of 1024 projection rows), RoPE, causal attention, and the
partial output projection y_g = attn_g @ wo[:, g*512:(g+1)*512].T.  The host
sums the two head-group partials per batch element.

Layout: scores are computed TRANSPOSED (k on partitions, q on the free
axis): exp() is elementwise, the denominator comes from M=1 ones-matmuls
(col-tiled into spare PE array columns), and exp(scores^T) feeds the PV
matmul directly as the moving operand.

Performance structure (v2):
 - Score matmuls for a head PAIR are row-tiled (tile_position (0,0)/(64,0)),
   contracting both 64-dim heads in one 128-partition moving stream: 2x.
 - PV matmuls for the pair are col-tiled ((0,0)/(0,64)) into one PSUM bank.
 - Softmax denominators accumulate in a PSUM bank via M=1 matmuls of a ones
   column against ex, 4-way col-tiled two-kb-at-a-time (parity rows).
 - exp() runs 1024 wide across the pair's two score banks in one ACT call.
 - The causal diagonal mask is applied AFTER exp by gpsimd affine_select
   (zero-fill) on the SBUF ex tile, keeping DVE out of the attention loop.
 - Projections/RoPE/transposes for s-block group g+1 are interleaved
   chunk-by-chunk with attention for q-group g, so the PE never idles while
   the ACT engine streams exps (keeps the HAM clock-gate at full rate) and
   the output projection of q-group g-1 fills the PE during later groups.
"""
import math
import os
from contextlib import ExitStack

import numpy as np
import ml_dtypes

import concourse.bass as bass
import concourse.tile as tile
from concourse import bacc, mybir
from concourse import masks
from concourse.bass_utils import run_bass_kernel_spmd

F32 = mybir.dt.float32
BF16 = mybir.dt.bfloat16

D = 1024          # d_model
NH = 16           # heads total
DK = 64           # head dim
S = 2048          # sequence
B = 4             # batch
THETA = 10000.0
HPG = 8           # heads per group (2 groups over 8 cores with 4 batches)
W = HPG * DK      # 512: local projection width
NSB = S // 128    # 16 s-blocks
NQG = 4           # 512-wide q groups
SCALE = 1.0 / math.sqrt(DK)

TRACE = bool(int(os.environ.get("KTRACE", "0")))

_cache = {}


def build_nc():
    sdt = BF16
    nc = bacc.Bacc(None, target_bir_lowering=False, debug=False)

    xt = nc.dram_tensor("xt", [D, S], sdt, kind="ExternalInput")
    wqt = nc.dram_tensor("wqt", [D, W], sdt, kind="ExternalInput")
    wkt = nc.dram_tensor("wkt", [D, W], sdt, kind="ExternalInput")
    wvt = nc.dram_tensor("wvt", [D, W], sdt, kind="ExternalInput")
    wot = nc.dram_tensor("wot", [W, D], sdt, kind="ExternalInput")
    cosb = nc.dram_tensor("cosb", [S, W // 2], F32, kind="ExternalInput")
    sinb = nc.dram_tensor("sinb", [S, W // 2], F32, kind="ExternalInput")
    yp = nc.dram_tensor("yp", [S, D], F32, kind="ExternalOutput")

    xt3 = xt[:].rearrange("(jo p) s -> p jo s", p=128)       # [128, 8, S]
    wqt3 = wqt[:].rearrange("(jo p) i -> p jo i", p=128)     # [128, 8, W]
    wkt3 = wkt[:].rearrange("(jo p) i -> p jo i", p=128)
    wvt3 = wvt[:].rearrange("(jo p) i -> p jo i", p=128)
    wot3 = wot[:].rearrange("(jo p) i -> p jo i", p=128)     # [128, 4, D]

    with tile.TileContext(nc, pool_alloc_mode="queue") as tc, \
            ExitStack() as ctx:
        persist = ctx.enter_context(tc.tile_pool(name="persist", bufs=1))
        ident = persist.tile([128, 128], F32, name="ident")
        masks.make_identity(nc, ident)
        ones = persist.tile([128, 1], sdt, name="ones")
        nc.gpsimd.memset(ones, 1.0)

        # persistent activations: q^T and k^T as 4 head-pair slabs
        # (rows 0:64 = head 2j dims [evens|odds], rows 64:128 = head 2j+1),
        # v s-major [128, 8*64].
        qT = [persist.tile([128, S], sdt, name=f"qT{j}") for j in range(4)]
        kT = [persist.tile([128, S], sdt, name=f"kT{j}") for j in range(4)]
        vt = [persist.tile([128, W], sdt, name=f"vt{i}") for i in range(NSB)]

        wp = ctx.enter_context(tc.tile_pool(name="wp", bufs=1))
        wq_s = wp.tile([128, 8, W], sdt, name="wq_s")
        nc.sync.dma_start(wq_s[:], wqt3[:])
        wk_s = wp.tile([128, 8, W], sdt, name="wk_s")
        wv_s = wp.tile([128, 8, W], sdt, name="wv_s")
        nc.gpsimd.dma_start(wk_s[:], wkt3[:])
        nc.gpsimd.dma_start(wv_s[:], wvt3[:])
        wo_s = wp.tile([128, 4, D], sdt, name="wo_s")
        nc.gpsimd.dma_start(wo_s[:], wot3[:])

        px = ctx.enter_context(tc.tile_pool(name="px", bufs=2))
        rp = ctx.enter_context(tc.tile_pool(name="rp", bufs=2))
        exp_p = ctx.enter_context(tc.tile_pool(name="exp_p", bufs=4))
        aqp = ctx.enter_context(tc.tile_pool(name="aqp", bufs=2))
        nrm = ctx.enter_context(tc.tile_pool(name="nrm", bufs=2))
        ytp = ctx.enter_context(tc.tile_pool(name="ytp", bufs=2))
        # PSUM pools are swapped between phases via this dict: super-step 0
        # (pure projection) gets fat scoped pools; the merged steps run with
        # pp(1) + tr(1) + sc(2x2) + pv(1) + den(1) = 8 banks
        pools = {}

        def rope(ps, outt, c3, s3):
            # ps: [128, W] PSUM (pre-RoPE proj, s-major, heads as
            # [evens(32) | odds(32)] blocks); outt: [128, W] SBUF
            pe = ps.rearrange("p (h eo c) -> p h eo c", eo=2, c=32)
            ein, oin = pe[:, :, 0, :], pe[:, :, 1, :]
            oe = outt.rearrange("p (h eo c) -> p h eo c", eo=2, c=32)
            eout, oout = oe[:, :, 0, :], oe[:, :, 1, :]
            # all 4 psum-reading muls first so the psum bank frees early
            ra = rp.tile([128, 8, 32], F32, name="ra", tag="ra")
            rb = rp.tile([128, 8, 32], F32, name="rb", tag="rb")
            rc = rp.tile([128, 8, 32], F32, name="rc", tag="rc")
            rd = rp.tile([128, 8, 32], F32, name="rd", tag="rd")
            nc.vector.tensor_mul(ra, ein, c3)
            nc.vector.tensor_mul(rb, oin, s3)
            nc.vector.tensor_mul(rc, ein, s3)
            nc.vector.tensor_mul(rd, oin, c3)
            nc.vector.tensor_sub(eout, ra, rb)
            nc.vector.tensor_add(oout, rc, rd)

        # ---------------- proj sub-chunk emitters -------------------------
        def proj_load(sb):
            s0 = sb * 128
            xs = px.tile([128, 8, 128], sdt, name="xs", tag="xs")
            nc.sync.dma_start(xs[:], xt3[:, :, s0:s0 + 128])
            cs = px.tile([128, W // 2], F32, name="cs", tag="cs")
            nc.sync.dma_start(cs[:], cosb[s0:s0 + 128, :])
            sn = px.tile([128, W // 2], F32, name="sn", tag="sn")
            nc.sync.dma_start(sn[:], sinb[s0:s0 + 128, :])
            return xs, cs, sn

        def proj_qk(which, sb, st):
            xs, cs, sn = st
            c3 = cs.rearrange("p (h c) -> p h c", c=32)
            s3 = sn.rearrange("p (h c) -> p h c", c=32)
            wsb = wq_s if which == "q" else wk_s
            pj = pools["pp"].tile([128, W], F32, name="pj", tag="pp")
            for jo in range(8):
                nc.tensor.matmul(pj[:], xs[:, jo, :], wsb[:, jo, :],
                                 start=(jo == 0), stop=(jo == 7))
            ro = rp.tile([128, W], F32, name=f"{which}_ro", tag=f"{which}ro")
            rope(pj, ro, c3, s3)
            return ro

        def proj_v(sb, st):
            xs, _, _ = st
            pj = pools["pp"].tile([128, W], F32, name="pj", tag="pp")
            for jo in range(8):
                nc.tensor.matmul(pj[:], xs[:, jo, :], wv_s[:, jo, :],
                                 start=(jo == 0), stop=(jo == 7))
            nc.scalar.copy(vt[sb][:], pj[:])

        def transp2(sb, pr, q_ro, k_ro):
            # transpose head-pair pr of q_ro and k_ro into the d-major slabs
            s0 = sb * 128
            c0 = pr * 128
            for srcb, dstl in ((q_ro, qT), (k_ro, kT)):
                ptr = pools["tr"].tile([128, 128], F32, name="ptr", tag="tr")
                nc.tensor.transpose(ptr[:], srcb[:, c0:c0 + 128], ident[:])
                nc.vector.tensor_copy(dstl[pr][:, s0:s0 + 128], ptr[:])

        # ---------------- attention chunk emitters ------------------------
        pair_state = {}

        def attn_pair_begin(qg, p):
            pvAB = pvp.tile([128, 512], F32, name="pvAB", tag="pv")
            den = dnp.tile([128, 512], F32, name="den", tag="den")
            pair_state[(qg, p)] = (pvAB, den, [])

        def attn_chunk(qg, p, kb):
            # scores + exp + mask for kb; PV/denominator for kb-1 (lagged one
            # step so the PV matmul's exp dependency is already satisfied
            # when it reaches the PE queue head)
            pvAB, den, exs = pair_state[(qg, p)]
            q0 = 512 * qg
            off = kb - 4 * qg
            c0 = 128 * max(off, 0)
            scAB = scp.tile([128, 1024], F32, name="scAB", tag="sc")
            nc.tensor.matmul(
                scAB[:, c0:512],
                kT[p][0:64, kb * 128:(kb + 1) * 128],
                qT[p][0:64, q0 + c0:q0 + 512],
                start=True, stop=True, tile_position=(0, 0))
            nc.tensor.matmul(
                scAB[:, 512 + c0:1024],
                kT[p][64:128, kb * 128:(kb + 1) * 128],
                qT[p][64:128, q0 + c0:q0 + 512],
                start=True, stop=True, tile_position=(64, 0))
            ex = exp_p.tile([128, 1024], sdt, name="ex", tag="ex")
            nc.scalar.activation(ex[:], scAB[:],
                                 mybir.ActivationFunctionType.Exp,
                                 scale=SCALE)
            if off >= 0:
                # zero the strictly-upper (k > q) triangle of the diagonal
                # 128-block of both heads, post-exp
                for base in (c0, 512 + c0):
                    nc.gpsimd.affine_select(
                        out=ex[:, base:base + 128], in_=ex[:, base:base + 128],
                        compare_op=mybir.AluOpType.is_ge, fill=0.0,
                        base=0, pattern=[[1, 128]], channel_multiplier=-1)
            exs.append((ex, c0))
            if kb >= 1:
                emit_pv_den(qg, p, kb - 1)

        def emit_pv_den(qg, p, kbx):
            pvAB, den, exs = pair_state[(qg, p)]
            nkb = 4 * qg + 4
            ex, c0 = exs[kbx]
            first, last = kbx == 0, kbx == nkb - 1
            nc.tensor.matmul(
                pvAB[0:64, c0:512], vt[kbx][:, 128 * p:128 * p + 64],
                ex[:, c0:512],
                start=first, stop=last, tile_position=(0, 0))
            nc.tensor.matmul(
                pvAB[64:128, c0:512], vt[kbx][:, 128 * p + 64:128 * p + 128],
                ex[:, 512 + c0:1024],
                start=first, stop=last, tile_position=(0, 64))
            # denominators: M=1 ones-matmuls, col-tiled into spare columns
            if qg == 0:
                nc.tensor.matmul(den[0:1, c0:512], ones[:], ex[:, c0:512],
                                 start=first, stop=last,
                                 tile_position=(0, 0))
                nc.tensor.matmul(den[32:33, c0:512], ones[:],
                                 ex[:, 512 + c0:1024],
                                 start=first, stop=last,
                                 tile_position=(0, 32))
            elif kbx % 2 == 1:
                exm, cm = exs[kbx - 1]
                fo = kbx == 1
                nc.tensor.matmul(den[0:1, cm:512], ones[:], exm[:, cm:512],
                                 start=fo, stop=last, tile_position=(0, 0))
                nc.tensor.matmul(den[32:33, cm:512], ones[:],
                                 exm[:, 512 + cm:1024],
                                 start=fo, stop=last, tile_position=(0, 32))
                nc.tensor.matmul(den[64:65, c0:512], ones[:], ex[:, c0:512],
                                 start=fo, stop=last, tile_position=(0, 64))
                nc.tensor.matmul(den[96:97, c0:512], ones[:],
                                 ex[:, 512 + c0:1024],
                                 start=fo, stop=last, tile_position=(0, 96))

        def attn_pair_end(qg, p, stage8):
            nkb = 4 * qg + 4
            emit_pv_den(qg, p, nkb - 1)
            pvAB, den, exs = pair_state.pop((qg, p))
            aq = aqp.tile([128, 512], sdt, name="aq", tag=f"aq{p}")
            nc.vector.tensor_copy(aq[:], pvAB[:])
            pair_state[("aq", qg, p)] = aq
            dtile = stage8 if qg != 3 else nrm.tile(
                [2, 512], F32, name="st2", tag="st2", bufs=2)
            r0off = 2 * p if qg != 3 else 0
            for row, h in ((0, 0), (32, 1)):
                if qg == 0:
                    dsb = nrm.tile([1, 512], F32, name="dsb", tag="dsb",
                                   bufs=4)
                    nc.vector.tensor_copy(dsb[:], den[row:row + 1, :])
                else:
                    tA = nrm.tile([1, 512], F32, name="tA", tag="tA", bufs=4)
                    nc.vector.tensor_copy(tA[:], den[row + 64:row + 65, :])
                    dsb = nrm.tile([1, 512], F32, name="dsb", tag="dsb",
                                   bufs=4)
                    nc.vector.tensor_add(dsb[:], den[row:row + 1, :], tA[:])
                nc.sync.dma_start(dtile[r0off + h:r0off + h + 1, :], dsb[:])
            if qg == 3:
                # pair-local normalize: overlaps the remaining pairs'
                # attention instead of serializing at the very end
                rall2 = nrm.tile([2, 512], F32, name="rall2", tag="rall2",
                                 bufs=2)
                nc.vector.reciprocal(rall2[:], dtile[:])
                _norm_heads(qg, ((p, 0, rall2, 0), (p, 1, rall2, 1)))

        def _norm_heads(qg, specs):
            for p, hh, rsrc, srow in specs:
                r0 = 64 * hh
                rsb = nrm.tile([1, 512], F32, name="rsb", tag="rsb", bufs=3)
                nc.sync.dma_start(rsb[:], rsrc[srow:srow + 1, :])
                rbc = nrm.tile([128, 512], F32, name="rbc", tag="rbc",
                               bufs=3)
                nc.gpsimd.partition_broadcast(rbc[:], rsb[:], channels=128)
                aq = pair_state[("aq", qg, p)]
                nc.vector.tensor_mul(aq[r0:r0 + 64, :], aq[r0:r0 + 64, :],
                                     rbc[r0:r0 + 64, :])

        def attn_norm(qg, stage8):
            rall8 = nrm.tile([8, 512], F32, name="rall8", tag="rall8")
            nc.vector.reciprocal(rall8[:], stage8[:])
            _norm_heads(qg, [(h // 2, h % 2, rall8, h) for h in range(HPG)])

        def outproj_half(qg, sbl, ih, yt):
            s0 = 512 * qg + sbl * 128
            py = pools["tr"].tile([128, 512], F32, name="py", tag="tr")
            for j in range(4):
                aq = pair_state[("aq", qg, j)]
                nc.tensor.matmul(py[:], aq[:, sbl * 128:(sbl + 1) * 128],
                                 wo_s[:, j, ih * 512:(ih + 1) * 512],
                                 start=(j == 0), stop=(j == 3))
            nc.vector.tensor_copy(yt[:, ih * 512:(ih + 1) * 512], py[:])
            if ih == 1:
                nc.sync.dma_start(yp[s0:s0 + 128, :], yt[:])

        # ---------------- merged schedule ---------------------------------
        def proj_items(g):
            items = []
            for sb in range(4 * g, 4 * g + 4):
                st_box = {}

                def load(sb=sb, st_box=st_box):
                    st_box["st"] = proj_load(sb)
                    st_box["q"] = proj_qk("q", sb, st_box["st"])
                items.append(load)
                def kproj(sb=sb, st_box=st_box):
                    st_box["k"] = proj_qk("k", sb, st_box["st"])
                items.append(kproj)
                items.append(lambda sb=sb, st_box=st_box:
                             proj_v(sb, st_box["st"]))
                for pr in range(4):
                    items.append(lambda sb=sb, pr=pr, st_box=st_box:
                                 transp2(sb, pr, st_box["q"], st_box["k"]))
            return items

        def outproj_items(qg):
            items = []
            for sbl in range(4):
                yt_box = {}

                def mk(qg=qg, sbl=sbl, ih=0, yt_box=yt_box):
                    yt_box["yt"] = ytp.tile([128, D], F32, name="yt",
                                            tag="yt")
                    outproj_half(qg, sbl, ih, yt_box["yt"])
                items.append(mk)
                items.append(lambda qg=qg, sbl=sbl, yt_box=yt_box:
                             outproj_half(qg, sbl, 1, yt_box["yt"]))
            return items

        def attn_items(qg, stage8):
            items = []
            nkb = 4 * qg + 4
            for p in range(4):
                items.append(lambda qg=qg, p=p: attn_pair_begin(qg, p))
                for kb in range(nkb):
                    items.append(lambda qg=qg, p=p, kb=kb:
                                 attn_chunk(qg, p, kb))
                items.append(lambda qg=qg, p=p, stage8=stage8:
                             attn_pair_end(qg, p, stage8))
            if qg != 3:
                items.append(lambda qg=qg, stage8=stage8:
                             attn_norm(qg, stage8))
            return items

        def interleave(main, filler):
            # spread filler items evenly through main items
            if not main:
                return filler
            out = []
            nf, nm = len(filler), len(main)
            fi = 0
            for i, m in enumerate(main):
                out.append(m)
                want = (i + 1) * nf // nm
                while fi < want:
                    out.append(filler[fi])
                    fi += 1
            out.extend(filler[fi:])
            return out

        # super-step 0: dense projection for group 0 with fat scoped PSUM
        # pools (the merged steps' pools are not yet allocated)
        with tc.tile_pool(name="pp0", bufs=3, space="PSUM") as pp0, \
                tc.tile_pool(name="tr0", bufs=4, space="PSUM") as tr0:
            pools["pp"], pools["tr"] = pp0, tr0
            for it in proj_items(0):
                it()

        pools["pp"] = ctx.enter_context(
            tc.tile_pool(name="pp", bufs=1, space="PSUM"))
        pools["tr"] = ctx.enter_context(
            tc.tile_pool(name="trp", bufs=1, space="PSUM"))
        scp = ctx.enter_context(tc.tile_pool(name="scp", bufs=2,
                                             space="PSUM"))
        pvp = ctx.enter_context(tc.tile_pool(name="pvp", bufs=1,
                                             space="PSUM"))
        dnp = ctx.enter_context(tc.tile_pool(name="dnp", bufs=1,
                                             space="PSUM"))

        for g in range(1, 5):
            filler = []
            if g < 4:
                filler.extend(proj_items(g))
            if g in (2, 3):
                filler.extend(outproj_items(g - 2))
            qg = g - 1
            stage8 = None
            if qg != 3:
                stage8 = nrm.tile([8, 512], F32, name=f"stage8_{qg}",
                                  tag="stage8")
            items = interleave(attn_items(qg, stage8), filler)
            if g == 4:
                # outproj(2) reserved for the tail: fills the PE while the
                # last pair's normalize chain completes
                items.extend(outproj_items(2))
            for it in items:
                it()
        for it in outproj_items(3):
            it()

    nc.compile()
    return nc


def _prep_inputs(x, token_positions, wq, wk, wv, wo):
    hdt = ml_dtypes.bfloat16
    # per-head permutation: [0,2,...,62, 1,3,...,63] (evens then odds)
    pi = np.concatenate([np.arange(0, DK, 2), np.arange(1, DK, 2)])
    perm = (np.arange(NH)[:, None] * DK + pi[None, :]).reshape(-1)
    wq_p = wq[perm, :]
    wk_p = wk[perm, :]

    pos = np.asarray(token_positions).astype(np.float32)
    thetas = (1.0 / (THETA ** (2.0 * np.arange(DK // 2, dtype=np.float32)
                               / DK))).astype(np.float32)
    ang = np.outer(pos, thetas).astype(np.float32)          # [S, 32]
    cos = np.tile(np.cos(ang), (1, HPG)).astype(np.float32)  # [S, 256]
    sin = np.tile(np.sin(ang), (1, HPG)).astype(np.float32)

    in_maps = []
    for core in range(8):
        b, g = core // 2, core % 2
        gs = slice(g * W, (g + 1) * W)
        in_maps.append({
            "xt": np.ascontiguousarray(x[b].T).astype(hdt),
            "wqt": np.ascontiguousarray(wq_p[gs, :].T).astype(hdt),
            "wkt": np.ascontiguousarray(wk_p[gs, :].T).astype(hdt),
            "wvt": np.ascontiguousarray(wv[gs, :].T).astype(hdt),
            "wot": np.ascontiguousarray(wo[:, gs].T).astype(hdt),
            "cosb": cos,
            "sinb": sin,
        })
    return in_maps


last_exec_time_ns = None
MODE = "bf16"


def _install_ntff_hook_shim():
    """This image's antenv lacks axon_hooks; wire the ctypes NTFF hook from
    trn_agent_boot so trace=True yields HW exec times."""
    import sys as _sys
    import types as _types
    try:
        from antenv import axon_hooks  # noqa: F401
        return
    except ImportError:
        pass
    from trn_agent_boot.trn_boot import _ntff_profile_via_ctypes
    hook = _ntff_profile_via_ctypes("/opt/axon/libaxon_pjrt.so")
    mod = _types.ModuleType("antenv.axon_hooks")
    mod.get_axon_ntff_profile_hook = lambda: hook
    _sys.modules["antenv.axon_hooks"] = mod


def kernel(x, token_positions, wq, wk, wv, wo):
    global last_exec_time_ns
    x = np.asarray(x, dtype=np.float32)
    token_positions = np.asarray(token_positions)
    wq = np.asarray(wq, dtype=np.float32)
    wk = np.asarray(wk, dtype=np.float32)
    wv = np.asarray(wv, dtype=np.float32)
    wo = np.asarray(wo, dtype=np.float32)

    if "nc" not in _cache:
        _cache["nc"] = build_nc()
    nc = _cache["nc"]

    in_maps = _prep_inputs(x, token_positions, wq, wk, wv, wo)
    res = None
    if TRACE:
        try:
            _install_ntff_hook_shim()
            res = run_bass_kernel_spmd(nc, in_maps, list(range(8)),
                                       trace=True,
                                       trace_cores=list(range(8)))
        except Exception as e:  # profiling must never sink correctness
            print(f"trace run failed ({type(e).__name__}: {e}); "
                  f"retrying untraced")
            res = None
    if res is None:
        res = run_bass_kernel_spmd(nc, in_maps, list(range(8)))
    last_exec_time_ns = res.exec_time_ns

    out = np.empty((B, S, D), dtype=np.float32)
    for b in range(B):
        out[b] = res.results[2 * b]["yp"] + res.results[2 * b + 1]["yp"]
    return out


# revision 12
# speedup vs baseline: 1.1904x; 1.0022x over previous
"""Causal multi-head self-attention with RoPE on 8 Trainium2 NeuronCores.

Sharding: core = (batch b, head-group g) with b = core//2, g = core%2.
Each core computes QKV projections for its batch element restricted to its
8 heads (512 of 1024 projection rows), RoPE, causal attention, and the
partial output projection y_g = attn_g @ wo[:, g*512:(g+1)*512].T.  The host
sums the two head-group partials per batch element.

Layout: scores are computed TRANSPOSED (k on partitions, q on the free
axis): exp() is elementwise, the denominator comes from M=1 ones-matmuls
(col-tiled into spare PE array columns), and exp(scores^T) feeds the PV
matmul directly as the moving operand.

Performance structure (v2):
 - Score matmuls for a head PAIR are row-tiled (tile_position (0,0)/(64,0)),
   contracting both 64-dim heads in one 128-partition moving stream: 2x.
 - PV matmuls for the pair are col-tiled ((0,0)/(0,64)) into one PSUM bank.
 - Softmax denominators accumulate in a PSUM bank via M=1 matmuls of a ones
   column against ex, 4-way col-tiled two-kb-at-a-time (parity rows).
 - exp() runs 1024 wide across the pair's two score banks in one ACT call.
 - The causal diagonal mask is applied AFTER exp by gpsimd affine_select
   (zero-fill) on the SBUF ex tile, keeping DVE out of the attention loop.
 - Projections/RoPE/transposes for s-block group g+1 are interleaved
   chunk-by-chunk with attention for q-group g, so the PE never idles while
   the ACT engine streams exps (keeps the HAM clock-gate at full rate) and
   the output projection of q-group g-1 fills the PE during later groups.
"""
import math
import os
from contextlib import ExitStack

import numpy as np
import ml_dtypes

import concourse.bass as bass
import concourse.tile as tile
from concourse import bacc, mybir
from concourse import masks
from concourse.bass_utils import run_bass_kernel_spmd

F32 = mybir.dt.float32
BF16 = mybir.dt.bfloat16

D = 1024          # d_model
NH = 16           # heads total
DK = 64           # head dim
S = 2048          # sequence
B = 4             # batch
THETA = 10000.0
HPG = 8           # heads per group (2 groups over 8 cores with 4 batches)
W = HPG * DK      # 512: local projection width
NSB = S // 128    # 16 s-blocks
NQG = 4           # 512-wide q groups
SCALE = 1.0 / math.sqrt(DK)

TRACE = bool(int(os.environ.get("KTRACE", "0")))

_cache = {}


def build_nc():
    sdt = BF16
    nc = bacc.Bacc(None, target_bir_lowering=False, debug=False)

    xt = nc.dram_tensor("xt", [D, S], sdt, kind="ExternalInput")
    wqt = nc.dram_tensor("wqt", [D, W], sdt, kind="ExternalInput")
    wkt = nc.dram_tensor("wkt", [D, W], sdt, kind="ExternalInput")
    wvt = nc.dram_tensor("wvt", [D, W], sdt, kind="ExternalInput")
    wot = nc.dram_tensor("wot", [W, D], sdt, kind="ExternalInput")
    cosb = nc.dram_tensor("cosb", [S, W // 2], F32, kind="ExternalInput")
    sinb = nc.dram_tensor("sinb", [S, W // 2], F32, kind="ExternalInput")
    yp = nc.dram_tensor("yp", [S, D], F32, kind="ExternalOutput")

    xt3 = xt[:].rearrange("(jo p) s -> p jo s", p=128)       # [128, 8, S]
    wqt3 = wqt[:].rearrange("(jo p) i -> p jo i", p=128)     # [128, 8, W]
    wkt3 = wkt[:].rearrange("(jo p) i -> p jo i", p=128)
    wvt3 = wvt[:].rearrange("(jo p) i -> p jo i", p=128)
    wot3 = wot[:].rearrange("(jo p) i -> p jo i", p=128)     # [128, 4, D]

    with tile.TileContext(nc, pool_alloc_mode="queue") as tc, \
            ExitStack() as ctx:
        persist = ctx.enter_context(tc.tile_pool(name="persist", bufs=1))
        ident = persist.tile([128, 128], F32, name="ident")
        masks.make_identity(nc, ident)
        ones = persist.tile([128, 1], sdt, name="ones")
        nc.gpsimd.memset(ones, 1.0)

        # persistent activations: q^T and k^T as 4 head-pair slabs
        # (rows 0:64 = head 2j dims [evens|odds], rows 64:128 = head 2j+1),
        # v s-major [128, 8*64].
        qT = [persist.tile([128, S], sdt, name=f"qT{j}") for j in range(4)]
        kT = [persist.tile([128, S], sdt, name=f"kT{j}") for j in range(4)]
        vt = [persist.tile([128, W], sdt, name=f"vt{i}") for i in range(NSB)]

        wp = ctx.enter_context(tc.tile_pool(name="wp", bufs=1))
        wq_s = wp.tile([128, 8, W], sdt, name="wq_s")
        nc.sync.dma_start(wq_s[:], wqt3[:])
        wk_s = wp.tile([128, 8, W], sdt, name="wk_s")
        wv_s = wp.tile([128, 8, W], sdt, name="wv_s")
        nc.gpsimd.dma_start(wk_s[:], wkt3[:])
        nc.gpsimd.dma_start(wv_s[:], wvt3[:])
        wo_s = wp.tile([128, 4, D], sdt, name="wo_s")
        nc.gpsimd.dma_start(wo_s[:], wot3[:])

        px = ctx.enter_context(tc.tile_pool(name="px", bufs=2))
        rp = ctx.enter_context(tc.tile_pool(name="rp", bufs=2))
        exp_p = ctx.enter_context(tc.tile_pool(name="exp_p", bufs=4))
        aqp = ctx.enter_context(tc.tile_pool(name="aqp", bufs=2))
        nrm = ctx.enter_context(tc.tile_pool(name="nrm", bufs=2))
        ytp = ctx.enter_context(tc.tile_pool(name="ytp", bufs=2))
        # PSUM pools are swapped between phases via this dict: super-step 0
        # (pure projection) gets fat scoped pools; the merged steps run with
        # pp(1) + tr(1) + sc(2x2) + pv(1) + den(1) = 8 banks
        pools = {}

        def rope(ps, outt, c3, s3):
            # ps: [128, W] PSUM (pre-RoPE proj, s-major, heads as
            # [evens(32) | odds(32)] blocks); outt: [128, W] SBUF
            pe = ps.rearrange("p (h eo c) -> p h eo c", eo=2, c=32)
            ein, oin = pe[:, :, 0, :], pe[:, :, 1, :]
            oe = outt.rearrange("p (h eo c) -> p h eo c", eo=2, c=32)
            eout, oout = oe[:, :, 0, :], oe[:, :, 1, :]
            # all 4 psum-reading muls first so the psum bank frees early
            ra = rp.tile([128, 8, 32], F32, name="ra", tag="ra")
            rb = rp.tile([128, 8, 32], F32, name="rb", tag="rb")
            rc = rp.tile([128, 8, 32], F32, name="rc", tag="rc")
            rd = rp.tile([128, 8, 32], F32, name="rd", tag="rd")
            nc.vector.tensor_mul(ra, ein, c3)
            nc.vector.tensor_mul(rb, oin, s3)
            nc.vector.tensor_mul(rc, ein, s3)
            nc.vector.tensor_mul(rd, oin, c3)
            nc.vector.tensor_sub(eout, ra, rb)
            nc.vector.tensor_add(oout, rc, rd)

        # ---------------- proj sub-chunk emitters -------------------------
        def proj_load(sb):
            s0 = sb * 128
            xs = px.tile([128, 8, 128], sdt, name="xs", tag="xs")
            nc.sync.dma_start(xs[:], xt3[:, :, s0:s0 + 128])
            cs = px.tile([128, W // 2], F32, name="cs", tag="cs")
            nc.sync.dma_start(cs[:], cosb[s0:s0 + 128, :])
            sn = px.tile([128, W // 2], F32, name="sn", tag="sn")
            nc.sync.dma_start(sn[:], sinb[s0:s0 + 128, :])
            return xs, cs, sn

        def proj_qk(which, sb, st):
            xs, cs, sn = st
            c3 = cs.rearrange("p (h c) -> p h c", c=32)
            s3 = sn.rearrange("p (h c) -> p h c", c=32)
            wsb = wq_s if which == "q" else wk_s
            pj = pools["pp"].tile([128, W], F32, name="pj", tag="pp")
            for jo in range(8):
                nc.tensor.matmul(pj[:], xs[:, jo, :], wsb[:, jo, :],
                                 start=(jo == 0), stop=(jo == 7))
            ro = rp.tile([128, W], F32, name=f"{which}_ro", tag=f"{which}ro")
            rope(pj, ro, c3, s3)
            return ro

        def proj_v(sb, st):
            xs, _, _ = st
            pj = pools["pp"].tile([128, W], F32, name="pj", tag="pp")
            for jo in range(8):
                nc.tensor.matmul(pj[:], xs[:, jo, :], wv_s[:, jo, :],
                                 start=(jo == 0), stop=(jo == 7))
            nc.scalar.copy(vt[sb][:], pj[:])

        def transp2(sb, pr, q_ro, k_ro):
            # transpose head-pair pr of q_ro and k_ro into the d-major slabs
            s0 = sb * 128
            c0 = pr * 128
            for srcb, dstl in ((q_ro, qT), (k_ro, kT)):
                ptr = pools["tr"].tile([128, 128], F32, name="ptr", tag="tr")
                nc.tensor.transpose(ptr[:], srcb[:, c0:c0 + 128], ident[:])
                nc.vector.tensor_copy(dstl[pr][:, s0:s0 + 128], ptr[:])

        # ---------------- attention chunk emitters ------------------------
        pair_state = {}

        def attn_pair_begin(qg, p):
            pvAB = pvp.tile([128, 512], F32, name="pvAB", tag="pv")
            den = dnp.tile([128, 512], F32, name="den", tag="den")
            pair_state[(qg, p)] = (pvAB, den, [])

        def attn_chunk(qg, p, kb):
            # scores + exp + mask for kb; PV/denominator for kb-1 (lagged one
            # step so the PV matmul's exp dependency is already satisfied
            # when it reaches the PE queue head)
            pvAB, den, exs = pair_state[(qg, p)]
            q0 = 512 * qg
            off = kb - 4 * qg
            c0 = 128 * max(off, 0)
            scAB = scp.tile([128, 1024], F32, name="scAB", tag="sc")
            nc.tensor.matmul(
                scAB[:, c0:512],
                kT[p][0:64, kb * 128:(kb + 1) * 128],
                qT[p][0:64, q0 + c0:q0 + 512],
                start=True, stop=True, tile_position=(0, 0))
            nc.tensor.matmul(
                scAB[:, 512 + c0:1024],
                kT[p][64:128, kb * 128:(kb + 1) * 128],
                qT[p][64:128, q0 + c0:q0 + 512],
                start=True, stop=True, tile_position=(64, 0))
            ex = exp_p.tile([128, 1024], sdt, name="ex", tag="ex")
            nc.scalar.activation(ex[:], scAB[:],
                                 mybir.ActivationFunctionType.Exp,
                                 scale=SCALE)
            if off >= 0:
                # zero the strictly-upper (k > q) triangle of the diagonal
                # 128-block of both heads, post-exp
                for base in (c0, 512 + c0):
                    nc.gpsimd.affine_select(
                        out=ex[:, base:base + 128], in_=ex[:, base:base + 128],
                        compare_op=mybir.AluOpType.is_ge, fill=0.0,
                        base=0, pattern=[[1, 128]], channel_multiplier=-1)
            exs.append((ex, c0))
            if kb >= 1:
                emit_pv_den(qg, p, kb - 1)

        def emit_pv_den(qg, p, kbx):
            pvAB, den, exs = pair_state[(qg, p)]
            nkb = 4 * qg + 4
            ex, c0 = exs[kbx]
            first, last = kbx == 0, kbx == nkb - 1
            nc.tensor.matmul(
                pvAB[0:64, c0:512], vt[kbx][:, 128 * p:128 * p + 64],
                ex[:, c0:512],
                start=first, stop=last, tile_position=(0, 0))
            nc.tensor.matmul(
                pvAB[64:128, c0:512], vt[kbx][:, 128 * p + 64:128 * p + 128],
                ex[:, 512 + c0:1024],
                start=first, stop=last, tile_position=(0, 64))
            # denominators: M=1 ones-matmuls, col-tiled into spare columns
            if qg == 0:
                nc.tensor.matmul(den[0:1, c0:512], ones[:], ex[:, c0:512],
                                 start=first, stop=last,
                                 tile_position=(0, 0))
                nc.tensor.matmul(den[32:33, c0:512], ones[:],
                                 ex[:, 512 + c0:1024],
                                 start=first, stop=last,
                                 tile_position=(0, 32))
            elif kbx % 2 == 1:
                exm, cm = exs[kbx - 1]
                fo = kbx == 1
                nc.tensor.matmul(den[0:1, cm:512], ones[:], exm[:, cm:512],
                                 start=fo, stop=last, tile_position=(0, 0))
                nc.tensor.matmul(den[32:33, cm:512], ones[:],
                                 exm[:, 512 + cm:1024],
                                 start=fo, stop=last, tile_position=(0, 32))
                nc.tensor.matmul(den[64:65, c0:512], ones[:], ex[:, c0:512],
                                 start=fo, stop=last, tile_position=(0, 64))
                nc.tensor.matmul(den[96:97, c0:512], ones[:],
                                 ex[:, 512 + c0:1024],
                                 start=fo, stop=last, tile_position=(0, 96))

        def attn_pair_end(qg, p, stage8):
            nkb = 4 * qg + 4
            emit_pv_den(qg, p, nkb - 1)
            pvAB, den, exs = pair_state.pop((qg, p))
            aq = aqp.tile([128, 512], sdt, name="aq", tag=f"aq{p}")
            if qg == 3:
                nc.scalar.copy(aq[:], pvAB[:])
            else:
                nc.vector.tensor_copy(aq[:], pvAB[:])
            pair_state[("aq", qg, p)] = aq
            dtile = stage8 if qg != 3 else nrm.tile(
                [2, 512], F32, name="st2", tag="st2", bufs=2)
            r0off = 2 * p if qg != 3 else 0
            for row, h in ((0, 0), (32, 1)):
                if qg == 0:
                    dsb = nrm.tile([1, 512], F32, name="dsb", tag="dsb",
                                   bufs=4)
                    nc.vector.tensor_copy(dsb[:], den[row:row + 1, :])
                else:
                    tA = nrm.tile([1, 512], F32, name="tA", tag="tA", bufs=4)
                    nc.vector.tensor_copy(tA[:], den[row + 64:row + 65, :])
                    dsb = nrm.tile([1, 512], F32, name="dsb", tag="dsb",
                                   bufs=4)
                    nc.vector.tensor_add(dsb[:], den[row:row + 1, :], tA[:])
                nc.sync.dma_start(dtile[r0off + h:r0off + h + 1, :], dsb[:])
            if qg == 3:
                # pair-local normalize: overlaps the remaining pairs'
                # attention instead of serializing at the very end
                rall2 = nrm.tile([2, 512], F32, name="rall2", tag="rall2",
                                 bufs=2)
                nc.vector.reciprocal(rall2[:], dtile[:])
                _norm_heads(qg, ((p, 0, rall2, 0), (p, 1, rall2, 1)))

        def _norm_heads(qg, specs):
            for p, hh, rsrc, srow in specs:
                r0 = 64 * hh
                rsb = nrm.tile([1, 512], F32, name="rsb", tag="rsb", bufs=3)
                nc.sync.dma_start(rsb[:], rsrc[srow:srow + 1, :])
                rbc = nrm.tile([128, 512], F32, name="rbc", tag="rbc",
                               bufs=3)
                nc.gpsimd.partition_broadcast(rbc[:], rsb[:], channels=128)
                aq = pair_state[("aq", qg, p)]
                nc.vector.tensor_mul(aq[r0:r0 + 64, :], aq[r0:r0 + 64, :],
                                     rbc[r0:r0 + 64, :])

        def attn_norm(qg, stage8):
            rall8 = nrm.tile([8, 512], F32, name="rall8", tag="rall8")
            nc.vector.reciprocal(rall8[:], stage8[:])
            _norm_heads(qg, [(h // 2, h % 2, rall8, h) for h in range(HPG)])

        def outproj_half(qg, sbl, ih, yt):
            s0 = 512 * qg + sbl * 128
            py = pools["tr"].tile([128, 512], F32, name="py", tag="tr")
            for j in range(4):
                aq = pair_state[("aq", qg, j)]
                nc.tensor.matmul(py[:], aq[:, sbl * 128:(sbl + 1) * 128],
                                 wo_s[:, j, ih * 512:(ih + 1) * 512],
                                 start=(j == 0), stop=(j == 3))
            # late q-groups evict via the (then idle) scalar engine so the
            # single tr PSUM slot is not gated on the DVE normalize chain
            if qg >= 2:
                nc.scalar.copy(yt[:, ih * 512:(ih + 1) * 512], py[:])
            else:
                nc.vector.tensor_copy(yt[:, ih * 512:(ih + 1) * 512], py[:])
            if ih == 1:
                nc.sync.dma_start(yp[s0:s0 + 128, :], yt[:])

        # ---------------- merged schedule ---------------------------------
        def proj_items(g):
            items = []
            for sb in range(4 * g, 4 * g + 4):
                st_box = {}

                def load(sb=sb, st_box=st_box):
                    st_box["st"] = proj_load(sb)
                    st_box["q"] = proj_qk("q", sb, st_box["st"])
                items.append(load)
                def kproj(sb=sb, st_box=st_box):
                    st_box["k"] = proj_qk("k", sb, st_box["st"])
                items.append(kproj)
                items.append(lambda sb=sb, st_box=st_box:
                             proj_v(sb, st_box["st"]))
                for pr in range(4):
                    items.append(lambda sb=sb, pr=pr, st_box=st_box:
                                 transp2(sb, pr, st_box["q"], st_box["k"]))
            return items

        def outproj_items(qg):
            items = []
            for sbl in range(4):
                yt_box = {}

                def mk(qg=qg, sbl=sbl, ih=0, yt_box=yt_box):
                    yt_box["yt"] = ytp.tile([128, D], F32, name="yt",
                                            tag="yt")
                    outproj_half(qg, sbl, ih, yt_box["yt"])
                items.append(mk)
                items.append(lambda qg=qg, sbl=sbl, yt_box=yt_box:
                             outproj_half(qg, sbl, 1, yt_box["yt"]))
            return items

        def attn_items(qg, stage8):
            items = []
            nkb = 4 * qg + 4
            for p in range(4):
                items.append(lambda qg=qg, p=p: attn_pair_begin(qg, p))
                for kb in range(nkb):
                    items.append(lambda qg=qg, p=p, kb=kb:
                                 attn_chunk(qg, p, kb))
                items.append(lambda qg=qg, p=p, stage8=stage8:
                             attn_pair_end(qg, p, stage8))
            if qg != 3:
                items.append(lambda qg=qg, stage8=stage8:
                             attn_norm(qg, stage8))
            return items

        def interleave(main, filler):
            # spread filler items evenly through main items
            if not main:
                return filler
            out = []
            nf, nm = len(filler), len(main)
            fi = 0
            for i, m in enumerate(main):
                out.append(m)
                want = (i + 1) * nf // nm
                while fi < want:
                    out.append(filler[fi])
                    fi += 1
            out.extend(filler[fi:])
            return out

        # super-step 0: dense projection for group 0 with fat scoped PSUM
        # pools (the merged steps' pools are not yet allocated)
        with tc.tile_pool(name="pp0", bufs=3, space="PSUM") as pp0, \
                tc.tile_pool(name="tr0", bufs=4, space="PSUM") as tr0:
            pools["pp"], pools["tr"] = pp0, tr0
            for it in proj_items(0):
                it()

        pools["pp"] = ctx.enter_context(
            tc.tile_pool(name="pp", bufs=1, space="PSUM"))
        pools["tr"] = ctx.enter_context(
            tc.tile_pool(name="trp", bufs=1, space="PSUM"))
        scp = ctx.enter_context(tc.tile_pool(name="scp", bufs=2,
                                             space="PSUM"))
        pvp = ctx.enter_context(tc.tile_pool(name="pvp", bufs=1,
                                             space="PSUM"))
        dnp = ctx.enter_context(tc.tile_pool(name="dnp", bufs=1,
                                             space="PSUM"))

        for g in range(1, 5):
            filler = []
            if g < 4:
                filler.extend(proj_items(g))
            if g in (2, 3):
                filler.extend(outproj_items(g - 2))
            qg = g - 1
            stage8 = None
            if qg != 3:
                stage8 = nrm.tile([8, 512], F32, name=f"stage8_{qg}",
                                  tag="stage8")
            items = interleave(attn_items(qg, stage8), filler)
            if g == 4:
                # outproj(2) reserved for the tail: fills the PE while the
                # last pair's normalize chain completes
                items.extend(outproj_items(2))
            for it in items:
                it()
        for it in outproj_items(3):
            it()

    nc.compile()
    return nc


def _prep_inputs(x, token_positions, wq, wk, wv, wo):
    hdt = ml_dtypes.bfloat16
    # per-head permutation: [0,2,...,62, 1,3,...,63] (evens then odds)
    pi = np.concatenate([np.arange(0, DK, 2), np.arange(1, DK, 2)])
    perm = (np.arange(NH)[:, None] * DK + pi[None, :]).reshape(-1)
    wq_p = wq[perm, :]
    wk_p = wk[perm, :]

    pos = np.asarray(token_positions).astype(np.float32)
    thetas = (1.0 / (THETA ** (2.0 * np.arange(DK // 2, dtype=np.float32)
                               / DK))).astype(np.float32)
    ang = np.outer(pos, thetas).astype(np.float32)          # [S, 32]
    cos = np.tile(np.cos(ang), (1, HPG)).astype(np.float32)  # [S, 256]
    sin = np.tile(np.sin(ang), (1, HPG)).astype(np.float32)

    in_maps = []
    for core in range(8):
        b, g = core // 2, core % 2
        gs = slice(g * W, (g + 1) * W)
        in_maps.append({
            "xt": np.ascontiguousarray(x[b].T).astype(hdt),
            "wqt": np.ascontiguousarray(wq_p[gs, :].T).astype(hdt),
            "wkt": np.ascontiguousarray(wk_p[gs, :].T).astype(hdt),
            "wvt": np.ascontiguousarray(wv[gs, :].T).astype(hdt),
            "wot": np.ascontiguousarray(wo[:, gs].T).astype(hdt),
            "cosb": cos,
            "sinb": sin,
        })
    return in_maps


last_exec_time_ns = None
MODE = "bf16"


def _install_ntff_hook_shim():
    """This image's antenv lacks axon_hooks; wire the ctypes NTFF hook from
    trn_agent_boot so trace=True yields HW exec times."""
    import sys as _sys
    import types as _types
    try:
        from antenv import axon_hooks  # noqa: F401
        return
    except ImportError:
        pass
    from trn_agent_boot.trn_boot import _ntff_profile_via_ctypes
    hook = _ntff_profile_via_ctypes("/opt/axon/libaxon_pjrt.so")
    mod = _types.ModuleType("antenv.axon_hooks")
    mod.get_axon_ntff_profile_hook = lambda: hook
    _sys.modules["antenv.axon_hooks"] = mod


def kernel(x, token_positions, wq, wk, wv, wo):
    global last_exec_time_ns
    x = np.asarray(x, dtype=np.float32)
    token_positions = np.asarray(token_positions)
    wq = np.asarray(wq, dtype=np.float32)
    wk = np.asarray(wk, dtype=np.float32)
    wv = np.asarray(wv, dtype=np.float32)
    wo = np.asarray(wo, dtype=np.float32)

    if "nc" not in _cache:
        _cache["nc"] = build_nc()
    nc = _cache["nc"]

    in_maps = _prep_inputs(x, token_positions, wq, wk, wv, wo)
    res = None
    if TRACE:
        try:
            _install_ntff_hook_shim()
            res = run_bass_kernel_spmd(nc, in_maps, list(range(8)),
                                       trace=True,
                                       trace_cores=list(range(8)))
        except Exception as e:  # profiling must never sink correctness
            print(f"trace run failed ({type(e).__name__}: {e}); "
                  f"retrying untraced")
            res = None
    if res is None:
        res = run_bass_kernel_spmd(nc, in_maps, list(range(8)))
    last_exec_time_ns = res.exec_time_ns

    out = np.empty((B, S, D), dtype=np.float32)
    for b in range(B):
        out[b] = res.results[2 * b]["yp"] + res.results[2 * b + 1]["yp"]
    return out
